# revision 70
# baseline (speedup 1.0000x reference)
"""Trainium2 Bass kernel for nn_Attention_8366596292664.

Dense transformer block: qkv proj -> RoPE -> GQA causal attention ->
out proj -> RMSNorm.  B=4, S=2048, H=2048, 16 heads (hd=128), 4 KV heads.

Sharding: 8 cores = (4 batches) x (2 interleaved query-row parities).
Core (b, par) computes the block for query rows {par, par+2, ...} of
batch b.  Keys/values stay in natural order; the parity enters only
through the q-side data (xq, cos/sin tables) and the band edge masks.

Structure (v4):
  Phase A1: k/v projection for the core's OWN 2 kv heads only; the
    other 2 heads' k/v come from the parity partner via a pairwise
    AllGather (DRAM bounce buffers, replica groups [2b, 2b+1]).  The k
    region stages to DRAM as soon as its rope evicts finish so the
    collective fires right after the last v eviction.
  Phase A2: q projection of heads 0..NH_A-1 (PE) + RoPE (vector) +
    PSUM eviction (scalar), sized to cover the collective's ~40us
    flight plus the ~20us gpsimd-queue unpack of the gathered k/v.
  Phase B: 16 attention slots.  The remaining heads' q projections are
    cut into 4-matmul quarter-groups and interleaved one per score
    pair: the PE does filler work exactly where it would otherwise
    stall on exp freeing a score-psum buffer, and stays dense enough
    to hold its fast p-state.  Scores run band-pairs first (their
    gpsimd edge-mask chains get maximum slack) then full pairs; y-MMs
    trail the score stream by 4 pairs; the softmax denominator is
    pair-summed on vector and contracted with a ones matrix into PSUM
    one tile late (FLUSH_AT pairs into the next tile).  Exp on scalar;
    band pairs use a single strided two-block exp; B-side RoPE is
    emitted per-t-half to spread vector load.
  Phase C: out-proj + RMSNorm, PE-saturated; 12/16 w_proj tiles
    prefetched into the SBUF freed mid-B by the q-projection inputs.
"""

import numpy as np
import ml_dtypes

BF16 = ml_dtypes.bfloat16

# ---------------------------------------------------------------- config
P = 128          # partitions
HD = 128         # head dim
HH = HD // 2     # rope half
G = 4            # GQA group size

B = 4
S = 2048
H = 2048
N_CORES = 8

NH = H // HD          # 16 q heads
NKV = NH // G         # 4 kv heads
KVC = NKV * HD        # 512 kv columns
HT = H // P           # 16 h-tiles (contraction tiles)
S_LOC = S // 2        # 1024 local q rows per core
IT = 512              # i-tile (queries per score tile, = 1 psum bank fp32)
NT_I = S_LOC // IT    # 2 i-slots
SPAN = S // NT_I      # 1024 global rows per slot
JB = SPAN // P        # 8 j-tiles in the diagonal band of each slot
OT = 512              # output-proj column tile
NO = H // OT          # 4
WPA = 12              # wp h-tiles prefetched during phase B

DEDUP = True          # exchange k/v halves across parity pairs
KVL = 2 if DEDUP else NKV      # kv heads computed locally
KVCL = KVL * HD                # local kv columns (k; v the same)
XCH = KVL * S * 2              # exchanged columns (k + v)
FLUSH_AT = 3
NH_A = 10             # q heads projected in phase A2 (covers the AllGather);
                      # the rest are interleaved into phase B as PE filler

RMS_EPS = 1e-6
SCALE = 1.0 / float(np.sqrt(np.float32(HD)))

_CACHE = {}


# ---------------------------------------------------------------- device IR
def _build_nc():
    from contextlib import ExitStack

    import concourse.bacc as bacc
    import concourse.mybir as mybir
    import concourse.tile as tile

    dt = mybir.dt
    AF = mybir.ActivationFunctionType

    nc = bacc.Bacc("TRN2", target_bir_lowering=False, debug=False,
                   num_devices=N_CORES)

    xt_d = nc.dram_tensor("xt", [HT, P, S], dt.bfloat16, kind="ExternalInput")
    xq_d = nc.dram_tensor("xq", [HT, P, S_LOC], dt.bfloat16, kind="ExternalInput")
    wq_d = nc.dram_tensor("wq", [NH, P, HT, HD], dt.bfloat16, kind="ExternalInput")
    wk_d = nc.dram_tensor("wk", [KVL, P, HT, HD], dt.bfloat16, kind="ExternalInput")
    wv_d = nc.dram_tensor("wv", [P, HT, KVCL], dt.bfloat16, kind="ExternalInput")
    wp_d = nc.dram_tensor("wp", [HT, P, H], dt.bfloat16, kind="ExternalInput")
    qcos_d = nc.dram_tensor("qcos", [P, S_LOC], dt.bfloat16, kind="ExternalInput")
    qsin_d = nc.dram_tensor("qsin", [P, S_LOC], dt.bfloat16, kind="ExternalInput")
    kcos_d = nc.dram_tensor("kcos", [P, S], dt.bfloat16, kind="ExternalInput")
    ksin_d = nc.dram_tensor("ksin", [P, S], dt.bfloat16, kind="ExternalInput")
    edge_d = nc.dram_tensor("edge", [P, 2 * 64], dt.bfloat16, kind="ExternalInput")
    nw_d = nc.dram_tensor("nw", [P, H], dt.float32, kind="ExternalInput")
    out_d = nc.dram_tensor("out", [S_LOC, H], dt.float32, kind="ExternalOutput")

    with tile.TileContext(nc) as tc, ExitStack() as body:
        const = body.enter_context(tc.tile_pool(name="const", bufs=1))
        onesm = const.tile([P, P], dt.bfloat16)
        nc.vector.memset(onesm[:], 1.0)
        epsb = const.tile([P, 1], dt.float32)
        nc.vector.memset(epsb[:], RMS_EPS)
        edge = const.tile([P, 2 * 64], dt.bfloat16)

        # tensors that live through phases A+B (freed before phase C)
        s_ab = body.enter_context(ExitStack())
        abp = s_ab.enter_context(tc.tile_pool(name="abp", bufs=1))
        qcos = abp.tile([P, S_LOC], dt.bfloat16)
        qsin = abp.tile([P, S_LOC], dt.bfloat16)
        # with DEDUP, kT / vv / qta are allocated only after phase A1's
        # xt frees (they are first written at the AllGather unpack)
        kv_box = []
        qta_box = []
        if not DEDUP:
            kv_box.append(abp.tile([P, NKV * S], dt.bfloat16, name="kT"))
            kv_box.append(abp.tile([P, (S // P) * KVC], dt.bfloat16, name="vv"))

        # yT lives B..C; created first so it sits at the bottom of the
        # right-side pool stack (the q pools above it pop mid-phase-B)
        latep = body.enter_context(tc.tile_pool(name="latep", bufs=1, side="right"))
        yTa = latep.tile([P, (NH // 2) * S_LOC], dt.bfloat16)
        yTb = latep.tile([P, (NH // 2) * S_LOC], dt.bfloat16)

        def yt_slice(h, c0, c1):
            # two half-tiles so phase C's early head-chains don't inherit
            # a coarse dependency on the very last head's finalize
            tl = yTa if h < NH // 2 else yTb
            base = (h % (NH // 2)) * S_LOC
            return tl[:, base + c0 : base + c1]

        # q-projection inputs live on the right side, away from the SBUF
        # region the kv-phase tiles occupy and free: their DMAs must not
        # inherit false deps on the kv-wave matmuls.  Closed mid-phase-B.
        s_q = body.enter_context(ExitStack())
        xqp = s_q.enter_context(tc.tile_pool(name="xqp", bufs=1, side="right"))
        xq = xqp.tile([P, HT * S_LOC], dt.bfloat16)
        wqr = s_q.enter_context(tc.tile_pool(name="wqr", bufs=4, side="right"))
        rpq = s_q.enter_context(tc.tile_pool(name="rpq", bufs=2, side="right"))

        if DEDUP:
            dram = body.enter_context(tc.tile_pool(name="dram", bufs=1, space="DRAM"))
            cin = dram.tile([P, XCH], dt.bfloat16)
            cout = dram.tile([2 * P, XCH], dt.bfloat16)

        # ---------------- phase A1: local k/v projection ----------------
        with ExitStack() as phKV:
            xtp = phKV.enter_context(tc.tile_pool(name="xtp", bufs=1))
            xt = xtp.tile([P, HT * S], dt.bfloat16)
            wkp = phKV.enter_context(tc.tile_pool(name="wkp", bufs=1))
            wk = wkp.tile([P, KVL * HT * HD], dt.bfloat16)
            kcs = phKV.enter_context(tc.tile_pool(name="kcs", bufs=1))
            kcos = kcs.tile([P, S], dt.bfloat16)
            ksin = kcs.tile([P, S], dt.bfloat16)
            if DEDUP:
                stgp = phKV.enter_context(tc.tile_pool(name="stgp", bufs=1))
                kvstage = stgp.tile([P, XCH], dt.bfloat16)
            wvp = phKV.enter_context(tc.tile_pool(name="wvp", bufs=1))
            wv = wvp.tile([P, HT * KVCL], dt.bfloat16)

            # weights + first x tiles first: they gate the first matmul
            def wk_chunk(fk, hc, n=8):
                nc.sync.dma_start(
                    wk[
                        :, fk * HT * HD + hc * HD : fk * HT * HD + (hc + n) * HD
                    ].rearrange("p (t m) -> p t m", t=n),
                    wk_d.ap()[fk, :, hc : hc + n],
                )

            wk_chunk(0, 0)
            nc.sync.dma_start(xt[:, 0:S], xt_d.ap()[0])
            wk_chunk(1, 0)
            nc.sync.dma_start(xt[:, S : 2 * S], xt_d.ap()[1])
            wk_chunk(0, 8)
            wk_chunk(1, 8)
            nc.sync.dma_start(
                wv[:].rearrange("p (t f) -> p t f", t=HT), wv_d.ap()
            )
            for h in range(2, HT):
                nc.sync.dma_start(xt[:, h * S : (h + 1) * S], xt_d.ap()[h])
            nc.sync.dma_start(kcos[:], kcos_d.ap())
            nc.sync.dma_start(ksin[:], ksin_d.ap())
            for fk in range(2, KVL):
                nc.sync.dma_start(
                    wk[:, fk * HT * HD : (fk + 1) * HT * HD].rearrange(
                        "p (t m) -> p t m", t=HT
                    ),
                    wk_d.ap()[fk],
                )
            for h in range(HT):
                nc.sync.dma_start(
                    xq[:, h * S_LOC : (h + 1) * S_LOC], xq_d.ap()[h]
                )
            nc.sync.dma_start(qcos[:], qcos_d.ap())
            nc.sync.dma_start(qsin[:], qsin_d.ap())
            nc.sync.dma_start(edge[:], edge_d.ap())

            psA = phKV.enter_context(tc.tile_pool(name="psA", bufs=8, space="PSUM"))
            rpk = phKV.enter_context(tc.tile_pool(name="rpk", bufs=2))

            k_dst = kvstage if DEDUP else kv_box[0]

            def rope_evict_k(ps, fk, sc):
                c0 = fk * S + sc * IT
                cs = kcos[:, sc * IT : (sc + 1) * IT]
                sn = ksin[:, sc * IT : (sc + 1) * IT]
                stg = rpk.tile([P, IT], dt.bfloat16, name="stgk")
                nc.scalar.activation(stg[:], ps[:], AF.Copy)
                t1 = rpk.tile([HH, IT], dt.bfloat16, name="rt1k", tag="rt1k", bufs=1)
                t2 = rpk.tile([HH, IT], dt.bfloat16, name="rt2k", tag="rt2k", bufs=1)
                nc.vector.tensor_mul(t1[:], stg[0:HH, :], cs[0:HH, :])
                nc.vector.tensor_mul(t2[:], stg[HH:P, :], sn[HH:P, :])
                nc.vector.tensor_sub(k_dst[0:HH, c0 : c0 + IT], t1[:], t2[:])
                nc.vector.tensor_mul(t1[:], stg[HH:P, :], cs[HH:P, :])
                nc.vector.tensor_mul(t2[:], stg[0:HH, :], sn[0:HH, :])
                nc.vector.tensor_add(k_dst[HH:P, c0 : c0 + IT], t1[:], t2[:])

            # k wave: interleaved groups, h-outer (DMA-paced)
            groups8 = [(fk, sc) for fk in range(min(KVL, 2)) for sc in range(S // IT)]
            kps = [psA.tile([P, IT], dt.float32, name="aps") for _ in groups8]
            for h in range(HT):
                for g, (fk, sc) in enumerate(groups8):
                    nc.tensor.matmul(
                        kps[g][:],
                        wk[:, fk * HT * HD + h * HD : fk * HT * HD + (h + 1) * HD],
                        xt[:, h * S + sc * IT : h * S + (sc + 1) * IT],
                        start=(h == 0),
                        stop=(h == HT - 1),
                    )
            for g, (fk, sc) in enumerate(groups8):
                rope_evict_k(kps[g], fk, sc)

            # remaining k groups (non-dedup fallback), group-outer
            for fk in range(2, KVL):
                for sc in range(S // IT):
                    ps = psA.tile([P, IT], dt.float32, name="aps")
                    for h in range(HT):
                        nc.tensor.matmul(
                            ps[:],
                            wk[:, fk * HT * HD + h * HD : fk * HT * HD + (h + 1) * HD],
                            xt[:, h * S + sc * IT : h * S + (sc + 1) * IT],
                            start=(h == 0),
                            stop=(h == HT - 1),
                        )
                    rope_evict_k(ps, fk, sc)

            # v wave (natural [s, f] layout)
            if DEDUP:
                # two 128-row s-tiles side by side in one psum bank
                for svp in range(S // P // 2):
                    ps = psA.tile([P, 2 * KVCL], dt.float32, name="aps")
                    first = True
                    for h in range(HT):
                        for u in range(2):
                            sv = 2 * svp + u
                            nc.tensor.matmul(
                                ps[:, u * KVCL : (u + 1) * KVCL],
                                xt[:, h * S + sv * P : h * S + (sv + 1) * P],
                                wv[:, h * KVCL : (h + 1) * KVCL],
                                start=first,
                                stop=(h == HT - 1 and u == 1),
                                skip_group_check=True,
                            )
                            first = False
                    nc.scalar.activation(
                        kvstage[
                            :, KVL * S + svp * 2 * KVCL : KVL * S + (svp + 1) * 2 * KVCL
                        ],
                        ps[:],
                        AF.Copy,
                    )
            else:
                for sv in range(S // P):
                    ps = psA.tile([P, KVC], dt.float32, name="aps")
                    for h in range(HT):
                        nc.tensor.matmul(
                            ps[:],
                            xt[:, h * S + sv * P : h * S + (sv + 1) * P],
                            wv[:, h * KVC : (h + 1) * KVC],
                            start=(h == 0),
                            stop=(h == HT - 1),
                        )
                    nc.scalar.activation(
                        kv_box[1][:, sv * KVC : (sv + 1) * KVC], ps[:], AF.Copy
                    )

            if DEDUP:
                # stage out + exchange with the parity partner.  On the
                # gpsimd queue: on the in-order sync queue the collective
                # wait would starve phase A2's xq/wq streaming.  The k
                # region stages as soon as its rope evicts finish (~30us
                # before v), so the collective triggers right after the
                # last v eviction.
                nc.gpsimd.dma_start(
                    cin[:, 0 : KVL * S], kvstage[:, 0 : KVL * S]
                )
                nc.gpsimd.dma_start(
                    cin[:, KVL * S : XCH], kvstage[:, KVL * S : XCH]
                )
                nc.gpsimd.collective_compute(
                    "AllGather",
                    mybir.AluOpType.bypass,
                    replica_groups=[[0, 1], [2, 3], [4, 5], [6, 7]],
                    ins=[cin[:].opt()],
                    outs=[cout[:].opt()],
                )

        if DEDUP:
            # kT / vv take over the SBUF phase A1 just freed
            ktvp = s_ab.enter_context(tc.tile_pool(name="ktvp", bufs=1))
            kv_box.append(ktvp.tile([P, NKV * S], dt.bfloat16, name="kT"))
            kv_box.append(ktvp.tile([P, (S // P) * KVC], dt.bfloat16, name="vv"))
            kT, vv = kv_box
            # unpack (waits on the collective, still on the gpsimd queue):
            # k: heads {0,1} from even rank, {2,3} from odd rank
            nc.gpsimd.dma_start(kT[:, 0 : KVL * S], cout[0:P, 0 : KVL * S])
            nc.gpsimd.dma_start(
                kT[:, KVL * S : NKV * S], cout[P : 2 * P, 0 : KVL * S]
            )
            # v: [s, 4*HD] tiles; cols 0:256 even rank, 256:512 odd rank
            vv3 = vv[:].rearrange("p (s c) -> p s c", s=S // P)
            for r in range(2):
                nc.gpsimd.dma_start(
                    vv3[:, :, r * KVCL : (r + 1) * KVCL],
                    cout[
                        r * P : (r + 1) * P, KVL * S : XCH
                    ].rearrange("p (s c) -> p s c", s=S // P),
                )
        else:
            kT, vv = kv_box

        # ---------------- q projection helpers --------------------------
        qstate = {}
        psQ_box = []

        def q_group(hq, t, pspool, psname, psbufs, halves=(0, 1)):
            # (part of) one i-slot group of head hq's q projection
            if t == 0 and halves[0] == 0:
                wqt = wqr.tile([P, HT * HD], dt.bfloat16, name="wqt")
                nc.sync.dma_start(
                    wqt[:].rearrange("p (t m) -> p t m", t=HT), wq_d.ap()[hq]
                )
                stg = rpq.tile([P, S_LOC], dt.bfloat16, name="stgq",
                               tag="stgq", bufs=2)
                qstate[hq] = [wqt, stg, None]
            st = qstate[hq]
            if halves[0] == 0:
                st[2] = pspool.tile([P, IT], dt.float32, name=psname,
                                    tag=psname, bufs=psbufs)
            ps = st[2]
            wqt, stg = st[0], st[1]
            for u in halves:
                for h in range(u * (HT // 2), (u + 1) * (HT // 2)):
                    nc.tensor.matmul(
                        ps[:],
                        wqt[:, h * HD : (h + 1) * HD],
                        xq[:, h * S_LOC + t * IT : h * S_LOC + (t + 1) * IT],
                        start=(h == 0),
                        stop=(h == HT - 1),
                    )
            if halves[-1] == 1:
                if pspool is psQ_box[0]:
                    # phase A2: scalar engine is idle there
                    nc.scalar.activation(
                        stg[:, t * IT : (t + 1) * IT], ps[:], AF.Copy
                    )
                else:
                    # phase B: keep the eviction off the exp-laden ACT FIFO
                    nc.vector.tensor_copy(stg[:, t * IT : (t + 1) * IT], ps[:])

        def rope_finish(hq):
            _, stg, _ = qstate.pop(hq)
            qt = qta_box[0][:, hq * S_LOC : (hq + 1) * S_LOC]
            t1 = rpq.tile([HH, S_LOC], dt.bfloat16, name="rt1q", tag="rt1q", bufs=1)
            t2 = rpq.tile([HH, S_LOC], dt.bfloat16, name="rt2q", tag="rt2q", bufs=1)
            nc.vector.tensor_mul(t1[:], stg[0:HH, :], qcos[0:HH, :])
            nc.vector.tensor_mul(t2[:], stg[HH:P, :], qsin[HH:P, :])
            nc.vector.tensor_sub(qt[0:HH, :], t1[:], t2[:])
            nc.vector.tensor_mul(t1[:], stg[HH:P, :], qcos[HH:P, :])
            nc.vector.tensor_mul(t2[:], stg[0:HH, :], qsin[0:HH, :])
            nc.vector.tensor_add(qt[HH:P, :], t1[:], t2[:])

        # ---------------- phase A2: q projection, heads 0..NH_A-1 -------
        # (runs while the k/v AllGather is in flight)
        qtp = s_ab.enter_context(tc.tile_pool(name="qtp", bufs=1))
        qta_box.append(qtp.tile([P, NH * S_LOC], dt.bfloat16, name="qta"))
        with ExitStack() as phQ:
            psQ = phQ.enter_context(tc.tile_pool(name="psQ", bufs=4, space="PSUM"))
            psQ_box.append(psQ)
            for hq in range(NH_A):
                q_group(hq, 0, psQ, "qpsA", 4)
                q_group(hq, 1, psQ, "qpsA", 4)
                rope_finish(hq)

        # ---------------- phase B: attention ----------------------------
        # deferred per-(head,t) finalize: denominator matmuls + reciprocal +
        # yT normalization, emitted a couple pairs later so the in-order PE
        # queue never waits on the cross-engine reduction chain.
        pending = [None]

        def flush_pending():
            if pending[0] is not None:
                fin, pending[0] = pending[0], None
                fin()

        with ExitStack() as phB:
            wpa_box = []

            pB = phB.enter_context(tc.tile_pool(name="pB", bufs=1, space="PSUM"))
            prp = phB.enter_context(tc.tile_pool(name="prp", bufs=6))
            dsp = phB.enter_context(tc.tile_pool(name="dsp", bufs=12))
            recp = phB.enter_context(tc.tile_pool(name="recp", bufs=2))
            sps = phB.enter_context(tc.tile_pool(name="sps", bufs=2, space="PSUM"))

            edge2 = edge[:].rearrange("p (u v) -> p u v", u=2)
            EW = IT - 64  # stride between the two edge blocks of a pair

            def attention(hq, fillers=()):
                fillers = list(fillers)

                def pop_filler():
                    if fillers:
                        fillers.pop(0)()

                kvh = hq // G
                qt = qta_box[0][:, hq * S_LOC : (hq + 1) * S_LOC]
                for t in range(NT_I):
                    qsl = qt[:, t * IT : (t + 1) * IT]
                    yps = pB.tile([P, IT], dt.float32, name="yps", tag="yps", bufs=2)
                    dss = []
                    state = {"y_first": True, "pend": [], "defers": 0}

                    def consume_pair(info, last):
                        # y-MMs + denominator pair-sum, emitted two pairs
                        # late so their semaphores clear before the PE
                        # reaches them
                        pr, j_hi, j_lo, ohi, olo = info
                        ds = dsp.tile([P, IT], dt.bfloat16, name="ds")
                        if ohi:  # band pair: ragged union [olo, IT)
                            nc.vector.tensor_copy(
                                ds[:, olo:ohi], pr[:, IT + olo : IT + ohi]
                            )
                            nc.vector.tensor_add(
                                ds[:, ohi:IT], pr[:, ohi:IT], pr[:, IT + ohi : 2 * IT]
                            )
                        else:
                            nc.vector.tensor_add(
                                ds[:, 0:IT], pr[:, 0:IT], pr[:, IT : 2 * IT]
                            )
                        dss.append((ds, olo))
                        nc.tensor.matmul(
                            yps[:, ohi:IT],
                            vv[:, j_hi * KVC + kvh * HD : j_hi * KVC + (kvh + 1) * HD],
                            pr[:, ohi:IT],
                            start=state["y_first"],
                            stop=False,
                        )
                        state["y_first"] = False
                        nc.tensor.matmul(
                            yps[:, olo:IT],
                            vv[:, j_lo * KVC + kvh * HD : j_lo * KVC + (kvh + 1) * HD],
                            pr[:, IT + olo : 2 * IT],
                            start=False,
                            stop=last,
                        )

                    def defer(info):
                        state["pend"].append(info)
                        if len(state["pend"]) > 4:
                            consume_pair(state["pend"].pop(0), last=False)
                        state["defers"] += 1
                        if state["defers"] == FLUSH_AT:
                            # previous (head,t)'s finalize, a couple pairs
                            # into this tile's stream
                            flush_pending()
                        pop_filler()

                    # diagonal band tiles: ragged pairs
                    for bp in range(JB // 2):
                        jlo, jhi = 2 * bp, 2 * bp + 1
                        j_lo, j_hi = t * JB + jlo, t * JB + jhi
                        olo, ohi = 64 * jlo, 64 * jhi
                        sp = sps.tile([P, 2 * IT], dt.float32, name="sps")
                        nc.tensor.matmul(
                            sp[:, ohi:IT],
                            kT[:, kvh * S + j_hi * P : kvh * S + (j_hi + 1) * P],
                            qt[:, t * IT + ohi : (t + 1) * IT],
                            start=True,
                            stop=True,
                        )
                        nc.tensor.matmul(
                            sp[:, IT + olo : 2 * IT],
                            kT[:, kvh * S + j_lo * P : kvh * S + (j_lo + 1) * P],
                            qt[:, t * IT + olo : (t + 1) * IT],
                            start=True,
                            stop=True,
                        )
                        pr = prp.tile([P, 2 * IT], dt.bfloat16, name="pr")
                        # one strided exp covering [olo,IT) and [IT+olo,2IT):
                        # equal-width blocks IT apart; the 64 cols [olo,ohi)
                        # of member0 are unwritten junk that nothing reads
                        nc.scalar.activation(
                            pr[:, 0 : 2 * IT].rearrange(
                                "p (u v) -> p u v", u=2
                            )[:, :, olo:IT],
                            sp[:, 0 : 2 * IT].rearrange(
                                "p (u v) -> p u v", u=2
                            )[:, :, olo:IT],
                            AF.Exp, scale=SCALE,
                        )
                        # both 64-wide edge blocks ([ohi,+64) and [IT+olo,+64),
                        # stride EW apart) in one strided op; X shifts the
                        # window so the 2*EW base slice stays inside the tile
                        X = max(0, ohi - (2 * IT - 2 * EW))
                        eap = pr[:, ohi - X : ohi - X + 2 * EW].rearrange(
                            "p (u v) -> p u v", u=2
                        )[:, :, X : X + 64]
                        nc.gpsimd.tensor_mul(eap, eap, edge2)
                        defer((pr, j_hi, j_lo, ohi, olo))
                    # full past tiles, in pairs (no edge dependency:
                    # they drain the tile cleanly while band edges settle)
                    for jp in range(t * JB // 2):
                        j0 = 2 * jp
                        sp = sps.tile([P, 2 * IT], dt.float32, name="sps")
                        for u in (0, 1):
                            nc.tensor.matmul(
                                sp[:, u * IT : (u + 1) * IT],
                                kT[:, kvh * S + (j0 + u) * P : kvh * S + (j0 + u + 1) * P],
                                qsl,
                                start=True,
                                stop=True,
                            )
                        pr = prp.tile([P, 2 * IT], dt.bfloat16, name="pr")
                        nc.scalar.activation(
                            pr[:, 0 : 2 * IT], sp[:], AF.Exp, scale=SCALE
                        )
                        defer((pr, j0, j0 + 1, 0, 0))

                    while fillers and t == NT_I - 1:
                        fillers.pop(0)()
                    while state["pend"]:
                        consume_pair(
                            state["pend"].pop(0), last=not state["pend"]
                        )

                    def fin(hq=hq, t=t, yps=yps, dss=tuple(dss)):
                        dps = pB.tile([P, IT], dt.float32, name="dps",
                                      tag="dps", bufs=1)
                        for i, (ds, lo) in enumerate(dss):
                            nc.tensor.matmul(
                                dps[:, lo:IT], onesm[:], ds[:, lo:IT],
                                start=(i == 0), stop=(i == len(dss) - 1),
                            )
                        rec = recp.tile([P, IT], dt.float32, name="rec")
                        nc.vector.reciprocal_approx_fast(rec[:], dps[:])
                        nc.vector.tensor_mul(
                            yt_slice(hq, t * IT, (t + 1) * IT),
                            yps[:],
                            rec[:],
                        )

                    pending[0] = fin

            # B-side q projection of heads NH_A..15 as 4-matmul quarter
            # groups, interleaved one per pair inside the attention stream:
            # the PE does filler work exactly where it would otherwise
            # wait for exp to free a score-psum buffer.
            def q_quarters(hq, t, u):
                if t == 0 and u == 0:
                    wqt = wqr.tile([P, HT * HD], dt.bfloat16, name="wqt")
                    nc.sync.dma_start(
                        wqt[:].rearrange("p (t m) -> p t m", t=HT),
                        wq_d.ap()[hq],
                    )
                    stg = rpq.tile([P, S_LOC], dt.bfloat16, name="stgq",
                                   tag="stgq", bufs=2)
                    qstate[hq] = [wqt, stg, None]

                def quarter(h0, hq=hq, t=t, u=u):
                    st = qstate[hq]
                    if u == 0 and h0 == 0:
                        st[2] = pB.tile([P, IT], dt.float32, name="qpsB",
                                        tag="qpsB", bufs=1)
                    wqt, stg, ps = st
                    for h in range(u * 8 + h0, u * 8 + h0 + 4):
                        nc.tensor.matmul(
                            ps[:],
                            wqt[:, h * HD : (h + 1) * HD],
                            xq[:, h * S_LOC + t * IT : h * S_LOC + (t + 1) * IT],
                            start=(h == 0),
                            stop=(h == HT - 1),
                        )
                    if u == 1 and h0 == 4:
                        nc.vector.tensor_copy(
                            stg[:, t * IT : (t + 1) * IT], ps[:]
                        )
                        if t == 1:
                            rope_finish(hq)

                return [lambda q=quarter: q(0), lambda q=quarter: q(4)]

            qb = [
                (NH_A + i // 4, (i // 2) % 2, i % 2)
                for i in range((NH - NH_A) * 4)
            ]
            # two half-groups (4 quarters) per slot until exhausted; done
            # by slot 11 so the w_proj prefetch gets the freed xq space
            nxt = 0
            for s in range(NH):
                fillers = []
                for _ in range(2):
                    if nxt < len(qb):
                        fillers += q_quarters(*qb[nxt])
                        nxt += 1
                attention(s, fillers)
                if nxt == len(qb) and not wpa_box:
                    # xq / wq streams are done: release their SBUF and
                    # start the w_proj prefetch in the freed region
                    s_q.close()
                    wpap = body.enter_context(
                        tc.tile_pool(name="wpap", bufs=1, side="right")
                    )
                    wpa_box.append(
                        wpap.tile([P, WPA * H], dt.bfloat16, name="wpa")
                    )
                    for i in range(WPA):
                        nc.sync.dma_start(
                            wpa_box[0][:, i * H : (i + 1) * H], wp_d.ap()[i]
                        )
            flush_pending()
            wpa = wpa_box[0]
        s_ab.close()  # free kT / vv / qta before the projection phase

        # ---------------- phase C: out projection + rmsnorm -------------
        with ExitStack() as phC:
            wpbp = phC.enter_context(tc.tile_pool(name="wpbp", bufs=1))
            wpc = wpbp.tile([P, (HT - WPA) * H], dt.bfloat16)
            for i in range(HT - WPA):
                nc.sync.dma_start(
                    wpc[:, i * H : (i + 1) * H], wp_d.ap()[WPA + i]
                )
            nwp = phC.enter_context(tc.tile_pool(name="nwp", bufs=1))
            nw = nwp.tile([P, H], dt.float32)
            nc.sync.dma_start(nw[:], nw_d.ap())

            outp = phC.enter_context(tc.tile_pool(name="outp", bufs=3))
            sqp = phC.enter_context(tc.tile_pool(name="sqp", bufs=3))
            smp = phC.enter_context(tc.tile_pool(name="smp", bufs=2))
            po = phC.enter_context(tc.tile_pool(name="po", bufs=8, space="PSUM"))

            mult = mybir.AluOpType.mult

            nslice = S_LOC // P
            for sl in range(nslice):
                # last slice runs o-outer so the norm chain pipelines with
                # the matmuls and the tail after the final matmul is short
                o_outer = sl == nslice - 1
                pso = [po.tile([P, OT], dt.float32, name="pso") for _ in range(NO)]
                ot = outp.tile([P, H], dt.float32, name="ot")
                ssqs = []

                def chunk_post(o):
                    sq = sqp.tile([P, OT], dt.float32, name="sq")
                    sso = smp.tile([P, 1], dt.float32, name="sso", tag="sso", bufs=8)
                    nc.scalar.activation(
                        sq[:], pso[o][:], AF.Square, accum_out=sso[:]
                    )
                    nc.scalar.activation(
                        ot[:, o * OT : (o + 1) * OT], pso[o][:], AF.Copy
                    )
                    ssqs.append(sso)

                lhss = [
                    yt_slice(h, sl * P, (sl + 1) * P)
                    for h in range(HT)
                ]
                def wslice(h, o):
                    if h < WPA:
                        return wpa[:, h * H + o * OT : h * H + (o + 1) * OT]
                    hh = h - WPA
                    return wpc[:, hh * H + o * OT : hh * H + (o + 1) * OT]

                if o_outer:
                    for o in range(NO):
                        for h in range(HT):
                            nc.tensor.matmul(
                                pso[o][:],
                                lhss[h],
                                wslice(h, o),
                                start=(h == 0),
                                stop=(h == HT - 1),
                            )
                        chunk_post(o)
                else:
                    for h in range(HT):
                        for o in range(NO):
                            nc.tensor.matmul(
                                pso[o][:],
                                lhss[h],
                                wslice(h, o),
                                start=(h == 0),
                                stop=(h == HT - 1),
                            )
                    for o in range(NO):
                        chunk_post(o)
                sa = smp.tile([P, 1], dt.float32, name="sa")
                sb = smp.tile([P, 1], dt.float32, name="sb")
                nc.vector.tensor_add(sa[:], ssqs[0][:], ssqs[1][:])
                nc.vector.tensor_add(sb[:], ssqs[2][:], ssqs[3][:])
                ssq = smp.tile([P, 1], dt.float32, name="ssq")
                nc.vector.tensor_add(ssq[:], sa[:], sb[:])
                rms = smp.tile([P, 1], dt.float32, name="rms")
                nc.scalar.activation(
                    rms[:], ssq[:], AF.Sqrt, bias=epsb[:], scale=1.0 / H
                )
                rr = smp.tile([P, 1], dt.float32, name="rr")
                nc.vector.reciprocal(rr[:], rms[:])
                for half in range(2):
                    for o in (2 * half, 2 * half + 1):
                        nc.vector.scalar_tensor_tensor(
                            ot[:, o * OT : (o + 1) * OT],
                            ot[:, o * OT : (o + 1) * OT],
                            rr[:],
                            nw[:, o * OT : (o + 1) * OT],
                            mult,
                            mult,
                        )
                    nc.sync.dma_start(
                        out_d.ap()[sl * P : (sl + 1) * P, half * H // 2 : (half + 1) * H // 2],
                        ot[:, half * H // 2 : (half + 1) * H // 2],
                    )

    nc.compile()
    return nc


# ---------------------------------------------------------------- host side
def _host_shared(w_attn, w_proj, norm_w):
    """Core-independent packed tensors."""
    f32 = np.float32

    def perm_halves(w):  # [H, n, HD] even/odd pairs -> halves
        return np.concatenate([w[..., 0::2], w[..., 1::2]], axis=-1)

    wq = perm_halves(w_attn[:, :H].reshape(H, NH, HD))
    wq = np.ascontiguousarray(
        wq.reshape(HT, P, NH, HD).transpose(2, 1, 0, 3)
    ).astype(BF16)
    wk = perm_halves(w_attn[:, H : H + KVC].reshape(H, NKV, HD))
    wk = np.ascontiguousarray(
        wk.reshape(HT, P, NKV, HD).transpose(2, 1, 0, 3)
    ).astype(BF16)
    wv = np.ascontiguousarray(
        w_attn[:, H + KVC :].reshape(HT, P, KVC)
    ).astype(BF16)
    wp = np.ascontiguousarray(w_proj.reshape(HT, P, H)).astype(BF16)

    p, f = np.meshgrid(np.arange(P), np.arange(64), indexing="ij")
    # parity 0: query col f = global row 2f vs key p (natural order)
    edge0 = (2 * f >= p).astype(BF16)
    # parity 1: query 2f+1 vs key p
    edge1 = (2 * f + 1 >= p).astype(BF16)
    # duplicated side by side: one strided op masks both blocks of a pair
    edge0 = np.ascontiguousarray(np.concatenate([edge0, edge0], axis=1))
    edge1 = np.ascontiguousarray(np.concatenate([edge1, edge1], axis=1))

    nw = np.ascontiguousarray(
        np.broadcast_to(norm_w.astype(f32), (P, H))
    )
    return wq, wk, wv, wp, (edge0, edge1), nw


def _cos_sin(pos):
    f32 = np.float32
    inv = 1.0 / (
        10000.0 ** (np.arange(0, HD, 2, dtype=f32) / f32(HD))
    )
    ang = inv[:, None].astype(f32) * pos[None, :].astype(f32)  # [HH, N]
    c, s = np.cos(ang).astype(BF16), np.sin(ang).astype(BF16)
    # duplicated across both partition halves (walrus wants equal base
    # partitions for SBUF tensor-tensor inputs)
    return (
        np.ascontiguousarray(np.concatenate([c, c], axis=0)),
        np.ascontiguousarray(np.concatenate([s, s], axis=0)),
    )


def make_in_maps(x, w_attn, w_proj, norm_w):
    x = np.asarray(x, dtype=np.float32)
    w_attn = np.asarray(w_attn, dtype=np.float32)
    w_proj = np.asarray(w_proj, dtype=np.float32)
    norm_w = np.asarray(norm_w, dtype=np.float32)

    wq, wk, wv, wp, (edge0, edge1), nw = _host_shared(w_attn, w_proj, norm_w)

    kc, ks = _cos_sin(np.arange(S, dtype=np.float32))
    qc0, qs0 = _cos_sin(2.0 * np.arange(S_LOC, dtype=np.float32))
    qc1, qs1 = _cos_sin(2.0 * np.arange(S_LOC, dtype=np.float32) + 1.0)

    in_maps = []
    for c in range(N_CORES):
        b, par = c // 2, c % 2
        xt = x[b].T.astype(BF16)
        # parity-packed contiguous copy for the q projection
        xq = np.ascontiguousarray(
            xt[:, par::2].reshape(HT, P, S_LOC)
        )
        xt = np.ascontiguousarray(xt.reshape(HT, P, S))
        if DEDUP:
            wk_c = np.ascontiguousarray(wk[2 * par : 2 * par + 2])
            wv_c = np.ascontiguousarray(
                wv[:, :, par * KVCL : (par + 1) * KVCL].transpose(1, 0, 2)
            )
        else:
            wk_c = wk
            wv_c = np.ascontiguousarray(wv.transpose(1, 0, 2))
        in_maps.append(
            {
                "xt": xt,
                "xq": xq,
                "wq": wq,
                "wk": wk_c,
                "wv": wv_c,
                "wp": wp,
                "qcos": qc1 if par else qc0,
                "qsin": qs1 if par else qs0,
                "kcos": kc,
                "ksin": ks,
                "edge": edge1 if par else edge0,
                "nw": nw,
            }
        )
    return in_maps


def assemble_out(results):
    out = np.empty((B, S, H), dtype=np.float32)
    for c in range(N_CORES):
        b, par = c // 2, c % 2
        out[b, par::2, :] = results[c]["out"]
    return out


def kernel(x, w_attn, w_proj, norm_w):
    from concourse import bass_utils

    if "nc" not in _CACHE:
        _CACHE["nc"] = _build_nc()
    nc = _CACHE["nc"]

    in_maps = make_in_maps(x, w_attn, w_proj, norm_w)
    res = bass_utils.run_bass_kernel_spmd(
        nc, in_maps, core_ids=list(range(N_CORES))
    )
    return assemble_out(res.results)


# revision 71
# speedup vs baseline: 1.1925x; 1.1925x over previous
"""Trainium2 Bass kernel for nn_Attention_8366596292664.

Dense transformer block: qkv proj -> RoPE -> GQA causal attention ->
out proj -> RMSNorm.  B=4, S=2048, H=2048, 16 heads (hd=128), 4 KV heads.

Sharding: 8 cores = (4 batches) x (2 interleaved query-row parities).
Core (b, par) computes the block for query rows {par, par+2, ...} of
batch b.  Keys/values stay in natural order; the parity enters only
through the q-side data (xq, cos/sin tables) and the band edge masks.

Structure (v4):
  Phase A1: k/v projection for the core's OWN 2 kv heads only; the
    other 2 heads' k/v come from the parity partner via a pairwise
    AllGather (DRAM bounce buffers, replica groups [2b, 2b+1]).  The k
    region stages to DRAM as soon as its rope evicts finish so the
    collective fires right after the last v eviction.
  Phase A2: q projection of heads 0..NH_A-1 (PE) + RoPE (vector) +
    PSUM eviction (scalar), sized to cover the collective's ~40us
    flight plus the ~20us gpsimd-queue unpack of the gathered k/v.
  Phase B: 16 attention slots.  The remaining heads' q projections are
    cut into 4-matmul quarter-groups and interleaved one per score
    pair: the PE does filler work exactly where it would otherwise
    stall on exp freeing a score-psum buffer, and stays dense enough
    to hold its fast p-state.  Scores run band-pairs first (their
    gpsimd edge-mask chains get maximum slack) then full pairs; y-MMs
    trail the score stream by 4 pairs; the softmax denominator is
    pair-summed on vector and contracted with a ones matrix into PSUM
    one tile late (FLUSH_AT pairs into the next tile).  Exp on scalar;
    band pairs use a single strided two-block exp; B-side RoPE is
    emitted per-t-half to spread vector load.
  Phase C: out-proj + RMSNorm, PE-saturated; 12/16 w_proj tiles
    prefetched into the SBUF freed mid-B by the q-projection inputs.
"""

import numpy as np
import ml_dtypes

BF16 = ml_dtypes.bfloat16

# ---------------------------------------------------------------- config
P = 128          # partitions
HD = 128         # head dim
HH = HD // 2     # rope half
G = 4            # GQA group size

B = 4
S = 2048
H = 2048
N_CORES = 8

NH = H // HD          # 16 q heads
NKV = NH // G         # 4 kv heads
KVC = NKV * HD        # 512 kv columns
HT = H // P           # 16 h-tiles (contraction tiles)
S_LOC = S // 2        # 1024 local q rows per core
IT = 512              # i-tile (queries per score tile, = 1 psum bank fp32)
NT_I = S_LOC // IT    # 2 i-slots
SPAN = S // NT_I      # 1024 global rows per slot
JB = SPAN // P        # 8 j-tiles in the diagonal band of each slot
OT = 512              # output-proj column tile
NO = H // OT          # 4
WPA = 12              # wp h-tiles prefetched during phase B

DEDUP = True          # exchange k/v halves across parity pairs
KVL = 2 if DEDUP else NKV      # kv heads computed locally
KVCL = KVL * HD                # local kv columns (k; v the same)
XCH = KVL * S * 2              # exchanged columns (k + v)
FLUSH_AT = 3
NH_A = 10             # q heads projected in phase A2 (covers the AllGather);
                      # the rest are interleaved into phase B as PE filler

RMS_EPS = 1e-6
SCALE = 1.0 / float(np.sqrt(np.float32(HD)))

_CACHE = {}


# ---------------------------------------------------------------- device IR
def _build_nc():
    from contextlib import ExitStack

    import concourse.bacc as bacc
    import concourse.mybir as mybir
    import concourse.tile as tile

    dt = mybir.dt
    AF = mybir.ActivationFunctionType

    nc = bacc.Bacc("TRN2", target_bir_lowering=False, debug=False,
                   num_devices=N_CORES)

    xt_d = nc.dram_tensor("xt", [HT, P, S], dt.bfloat16, kind="ExternalInput")
    xq_d = nc.dram_tensor("xq", [HT, P, S_LOC], dt.bfloat16, kind="ExternalInput")
    wq_d = nc.dram_tensor("wq", [NH, P, HT, HD], dt.bfloat16, kind="ExternalInput")
    wk_d = nc.dram_tensor("wk", [KVL, P, HT, HD], dt.bfloat16, kind="ExternalInput")
    wv_d = nc.dram_tensor("wv", [P, HT, KVCL], dt.bfloat16, kind="ExternalInput")
    wp_d = nc.dram_tensor("wp", [HT, P, H], dt.bfloat16, kind="ExternalInput")
    qcos_d = nc.dram_tensor("qcos", [P, S_LOC], dt.bfloat16, kind="ExternalInput")
    qsin_d = nc.dram_tensor("qsin", [P, S_LOC], dt.bfloat16, kind="ExternalInput")
    kcos_d = nc.dram_tensor("kcos", [P, S], dt.bfloat16, kind="ExternalInput")
    ksin_d = nc.dram_tensor("ksin", [P, S], dt.bfloat16, kind="ExternalInput")
    edge_d = nc.dram_tensor("edge", [P, 2 * 64], dt.bfloat16, kind="ExternalInput")
    nw_d = nc.dram_tensor("nw", [P, H], dt.float32, kind="ExternalInput")
    out_d = nc.dram_tensor("out", [S_LOC, H], dt.float32, kind="ExternalOutput")

    with tile.TileContext(nc) as tc, ExitStack() as body:
        const = body.enter_context(tc.tile_pool(name="const", bufs=1))
        onesm = const.tile([P, P], dt.bfloat16)
        nc.vector.memset(onesm[:], 1.0)
        epsb = const.tile([P, 1], dt.float32)
        nc.vector.memset(epsb[:], RMS_EPS)
        edge = const.tile([P, 2 * 64], dt.bfloat16)

        # tensors that live through phases A+B (freed before phase C)
        s_ab = body.enter_context(ExitStack())
        abp = s_ab.enter_context(tc.tile_pool(name="abp", bufs=1))
        qcos = abp.tile([P, S_LOC], dt.bfloat16)
        qsin = abp.tile([P, S_LOC], dt.bfloat16)
        # with DEDUP, kT / vv / qta are allocated only after phase A1's
        # xt frees (they are first written at the AllGather unpack)
        kv_box = []
        qta_box = []
        if not DEDUP:
            kv_box.append(abp.tile([P, NKV * S], dt.bfloat16, name="kT"))
            kv_box.append(abp.tile([P, (S // P) * KVC], dt.bfloat16, name="vv"))

        # yT lives B..C; created first so it sits at the bottom of the
        # right-side pool stack (the q pools above it pop mid-phase-B)
        latep = body.enter_context(tc.tile_pool(name="latep", bufs=1, side="right"))
        yT = latep.tile([P, NH * S_LOC], dt.bfloat16)

        # q-projection inputs live on the right side, away from the SBUF
        # region the kv-phase tiles occupy and free: their DMAs must not
        # inherit false deps on the kv-wave matmuls.  Closed mid-phase-B.
        s_q = body.enter_context(ExitStack())
        xqp = s_q.enter_context(tc.tile_pool(name="xqp", bufs=1, side="right"))
        xq = xqp.tile([P, HT * S_LOC], dt.bfloat16)
        wqr = s_q.enter_context(tc.tile_pool(name="wqr", bufs=4, side="right"))
        rpq = s_q.enter_context(tc.tile_pool(name="rpq", bufs=2, side="right"))

        if DEDUP:
            dram = body.enter_context(tc.tile_pool(name="dram", bufs=1, space="DRAM"))
            cin = dram.tile([P, XCH], dt.bfloat16)
            cout = dram.tile([2 * P, XCH], dt.bfloat16)

        # ---------------- phase A1: local k/v projection ----------------
        with ExitStack() as phKV:
            xtp = phKV.enter_context(tc.tile_pool(name="xtp", bufs=1))
            xt = xtp.tile([P, HT * S], dt.bfloat16)
            wkp = phKV.enter_context(tc.tile_pool(name="wkp", bufs=1))
            wk = wkp.tile([P, KVL * HT * HD], dt.bfloat16)
            kcs = phKV.enter_context(tc.tile_pool(name="kcs", bufs=1))
            kcos = kcs.tile([P, S], dt.bfloat16)
            ksin = kcs.tile([P, S], dt.bfloat16)
            if DEDUP:
                stgp = phKV.enter_context(tc.tile_pool(name="stgp", bufs=1))
                kvstage = stgp.tile([P, XCH], dt.bfloat16)
            wvp = phKV.enter_context(tc.tile_pool(name="wvp", bufs=1))
            wv = wvp.tile([P, HT * KVCL], dt.bfloat16)

            # weights + first x tiles first: they gate the first matmul
            def wk_chunk(fk, hc, n=8):
                nc.sync.dma_start(
                    wk[
                        :, fk * HT * HD + hc * HD : fk * HT * HD + (hc + n) * HD
                    ].rearrange("p (t m) -> p t m", t=n),
                    wk_d.ap()[fk, :, hc : hc + n],
                )

            wk_chunk(0, 0)
            nc.sync.dma_start(xt[:, 0:S], xt_d.ap()[0])
            wk_chunk(1, 0)
            nc.sync.dma_start(xt[:, S : 2 * S], xt_d.ap()[1])
            wk_chunk(0, 8)
            wk_chunk(1, 8)
            nc.sync.dma_start(
                wv[:].rearrange("p (t f) -> p t f", t=HT), wv_d.ap()
            )
            for h in range(2, HT):
                nc.sync.dma_start(xt[:, h * S : (h + 1) * S], xt_d.ap()[h])
            nc.sync.dma_start(kcos[:], kcos_d.ap())
            nc.sync.dma_start(ksin[:], ksin_d.ap())
            for fk in range(2, KVL):
                nc.sync.dma_start(
                    wk[:, fk * HT * HD : (fk + 1) * HT * HD].rearrange(
                        "p (t m) -> p t m", t=HT
                    ),
                    wk_d.ap()[fk],
                )
            for h in range(HT):
                nc.sync.dma_start(
                    xq[:, h * S_LOC : (h + 1) * S_LOC], xq_d.ap()[h]
                )
            nc.sync.dma_start(qcos[:], qcos_d.ap())
            nc.sync.dma_start(qsin[:], qsin_d.ap())
            nc.sync.dma_start(edge[:], edge_d.ap())

            psA = phKV.enter_context(tc.tile_pool(name="psA", bufs=8, space="PSUM"))
            rpk = phKV.enter_context(tc.tile_pool(name="rpk", bufs=2))

            k_dst = kvstage if DEDUP else kv_box[0]

            def rope_evict_k(ps, fk, sc):
                c0 = fk * S + sc * IT
                cs = kcos[:, sc * IT : (sc + 1) * IT]
                sn = ksin[:, sc * IT : (sc + 1) * IT]
                stg = rpk.tile([P, IT], dt.bfloat16, name="stgk")
                nc.scalar.activation(stg[:], ps[:], AF.Copy)
                t1 = rpk.tile([HH, IT], dt.bfloat16, name="rt1k", tag="rt1k", bufs=1)
                t2 = rpk.tile([HH, IT], dt.bfloat16, name="rt2k", tag="rt2k", bufs=1)
                nc.vector.tensor_mul(t1[:], stg[0:HH, :], cs[0:HH, :])
                nc.vector.tensor_mul(t2[:], stg[HH:P, :], sn[HH:P, :])
                nc.vector.tensor_sub(k_dst[0:HH, c0 : c0 + IT], t1[:], t2[:])
                nc.vector.tensor_mul(t1[:], stg[HH:P, :], cs[HH:P, :])
                nc.vector.tensor_mul(t2[:], stg[0:HH, :], sn[0:HH, :])
                nc.vector.tensor_add(k_dst[HH:P, c0 : c0 + IT], t1[:], t2[:])

            # k wave: interleaved groups, h-outer (DMA-paced)
            groups8 = [(fk, sc) for fk in range(min(KVL, 2)) for sc in range(S // IT)]
            kps = [psA.tile([P, IT], dt.float32, name="aps") for _ in groups8]
            for h in range(HT):
                for g, (fk, sc) in enumerate(groups8):
                    nc.tensor.matmul(
                        kps[g][:],
                        wk[:, fk * HT * HD + h * HD : fk * HT * HD + (h + 1) * HD],
                        xt[:, h * S + sc * IT : h * S + (sc + 1) * IT],
                        start=(h == 0),
                        stop=(h == HT - 1),
                    )
            for g, (fk, sc) in enumerate(groups8):
                rope_evict_k(kps[g], fk, sc)

            # remaining k groups (non-dedup fallback), group-outer
            for fk in range(2, KVL):
                for sc in range(S // IT):
                    ps = psA.tile([P, IT], dt.float32, name="aps")
                    for h in range(HT):
                        nc.tensor.matmul(
                            ps[:],
                            wk[:, fk * HT * HD + h * HD : fk * HT * HD + (h + 1) * HD],
                            xt[:, h * S + sc * IT : h * S + (sc + 1) * IT],
                            start=(h == 0),
                            stop=(h == HT - 1),
                        )
                    rope_evict_k(ps, fk, sc)

            # v wave (natural [s, f] layout)
            if DEDUP:
                # two 128-row s-tiles side by side in one psum bank
                for svp in range(S // P // 2):
                    ps = psA.tile([P, 2 * KVCL], dt.float32, name="aps")
                    first = True
                    for h in range(HT):
                        for u in range(2):
                            sv = 2 * svp + u
                            nc.tensor.matmul(
                                ps[:, u * KVCL : (u + 1) * KVCL],
                                xt[:, h * S + sv * P : h * S + (sv + 1) * P],
                                wv[:, h * KVCL : (h + 1) * KVCL],
                                start=first,
                                stop=(h == HT - 1 and u == 1),
                                skip_group_check=True,
                            )
                            first = False
                    nc.scalar.activation(
                        kvstage[
                            :, KVL * S + svp * 2 * KVCL : KVL * S + (svp + 1) * 2 * KVCL
                        ],
                        ps[:],
                        AF.Copy,
                    )
            else:
                for sv in range(S // P):
                    ps = psA.tile([P, KVC], dt.float32, name="aps")
                    for h in range(HT):
                        nc.tensor.matmul(
                            ps[:],
                            xt[:, h * S + sv * P : h * S + (sv + 1) * P],
                            wv[:, h * KVC : (h + 1) * KVC],
                            start=(h == 0),
                            stop=(h == HT - 1),
                        )
                    nc.scalar.activation(
                        kv_box[1][:, sv * KVC : (sv + 1) * KVC], ps[:], AF.Copy
                    )

            if DEDUP:
                # stage out + exchange with the parity partner.  On the
                # gpsimd queue: on the in-order sync queue the collective
                # wait would starve phase A2's xq/wq streaming.  The k
                # region stages as soon as its rope evicts finish (~30us
                # before v), so the collective triggers right after the
                # last v eviction.
                nc.gpsimd.dma_start(
                    cin[:, 0 : KVL * S], kvstage[:, 0 : KVL * S]
                )
                nc.gpsimd.dma_start(
                    cin[:, KVL * S : XCH], kvstage[:, KVL * S : XCH]
                )
                nc.gpsimd.collective_compute(
                    "AllGather",
                    mybir.AluOpType.bypass,
                    replica_groups=[[0, 1], [2, 3], [4, 5], [6, 7]],
                    ins=[cin[:].opt()],
                    outs=[cout[:].opt()],
                )

        if DEDUP:
            # kT / vv take over the SBUF phase A1 just freed
            ktvp = s_ab.enter_context(tc.tile_pool(name="ktvp", bufs=1))
            kv_box.append(ktvp.tile([P, NKV * S], dt.bfloat16, name="kT"))
            kv_box.append(ktvp.tile([P, (S // P) * KVC], dt.bfloat16, name="vv"))
            kT, vv = kv_box
            # unpack (waits on the collective, still on the gpsimd queue):
            # k: heads {0,1} from even rank, {2,3} from odd rank
            nc.gpsimd.dma_start(kT[:, 0 : KVL * S], cout[0:P, 0 : KVL * S])
            nc.gpsimd.dma_start(
                kT[:, KVL * S : NKV * S], cout[P : 2 * P, 0 : KVL * S]
            )
            # v: [s, 4*HD] tiles; cols 0:256 even rank, 256:512 odd rank
            vv3 = vv[:].rearrange("p (s c) -> p s c", s=S // P)
            for r in range(2):
                nc.gpsimd.dma_start(
                    vv3[:, :, r * KVCL : (r + 1) * KVCL],
                    cout[
                        r * P : (r + 1) * P, KVL * S : XCH
                    ].rearrange("p (s c) -> p s c", s=S // P),
                )
        else:
            kT, vv = kv_box

        # ---------------- q projection helpers --------------------------
        qstate = {}
        psQ_box = []

        def q_group(hq, t, pspool, psname, psbufs, halves=(0, 1)):
            # (part of) one i-slot group of head hq's q projection
            if t == 0 and halves[0] == 0:
                wqt = wqr.tile([P, HT * HD], dt.bfloat16, name="wqt")
                nc.sync.dma_start(
                    wqt[:].rearrange("p (t m) -> p t m", t=HT), wq_d.ap()[hq]
                )
                stg = rpq.tile([P, S_LOC], dt.bfloat16, name="stgq",
                               tag="stgq", bufs=2)
                qstate[hq] = [wqt, stg, None]
            st = qstate[hq]
            if halves[0] == 0:
                st[2] = pspool.tile([P, IT], dt.float32, name=psname,
                                    tag=psname, bufs=psbufs)
            ps = st[2]
            wqt, stg = st[0], st[1]
            for u in halves:
                for h in range(u * (HT // 2), (u + 1) * (HT // 2)):
                    nc.tensor.matmul(
                        ps[:],
                        wqt[:, h * HD : (h + 1) * HD],
                        xq[:, h * S_LOC + t * IT : h * S_LOC + (t + 1) * IT],
                        start=(h == 0),
                        stop=(h == HT - 1),
                    )
            if halves[-1] == 1:
                if pspool is psQ_box[0]:
                    # phase A2: scalar engine is idle there
                    nc.scalar.activation(
                        stg[:, t * IT : (t + 1) * IT], ps[:], AF.Copy
                    )
                else:
                    # phase B: keep the eviction off the exp-laden ACT FIFO
                    nc.vector.tensor_copy(stg[:, t * IT : (t + 1) * IT], ps[:])

        def rope_finish(hq):
            _, stg, _ = qstate.pop(hq)
            qt = qta_box[0][:, hq * S_LOC : (hq + 1) * S_LOC]
            t1 = rpq.tile([HH, S_LOC], dt.bfloat16, name="rt1q", tag="rt1q", bufs=1)
            t2 = rpq.tile([HH, S_LOC], dt.bfloat16, name="rt2q", tag="rt2q", bufs=1)
            nc.vector.tensor_mul(t1[:], stg[0:HH, :], qcos[0:HH, :])
            nc.vector.tensor_mul(t2[:], stg[HH:P, :], qsin[HH:P, :])
            nc.vector.tensor_sub(qt[0:HH, :], t1[:], t2[:])
            nc.vector.tensor_mul(t1[:], stg[HH:P, :], qcos[HH:P, :])
            nc.vector.tensor_mul(t2[:], stg[0:HH, :], qsin[0:HH, :])
            nc.vector.tensor_add(qt[HH:P, :], t1[:], t2[:])

        # ---------------- phase A2: q projection, heads 0..NH_A-1 -------
        # (runs while the k/v AllGather is in flight)
        qtp = s_ab.enter_context(tc.tile_pool(name="qtp", bufs=1))
        qta_box.append(qtp.tile([P, NH * S_LOC], dt.bfloat16, name="qta"))
        with ExitStack() as phQ:
            psQ = phQ.enter_context(tc.tile_pool(name="psQ", bufs=4, space="PSUM"))
            psQ_box.append(psQ)
            for hq in range(NH_A):
                q_group(hq, 0, psQ, "qpsA", 4)
                q_group(hq, 1, psQ, "qpsA", 4)
                rope_finish(hq)

        # ---------------- phase B: attention ----------------------------
        # deferred per-(head,t) finalize: denominator matmuls + reciprocal +
        # yT normalization, emitted a couple pairs later so the in-order PE
        # queue never waits on the cross-engine reduction chain.
        pending = [None]

        def flush_pending():
            if pending[0] is not None:
                fin, pending[0] = pending[0], None
                fin()

        with ExitStack() as phB:
            wpa_box = []

            pB = phB.enter_context(tc.tile_pool(name="pB", bufs=1, space="PSUM"))
            prp = phB.enter_context(tc.tile_pool(name="prp", bufs=6))
            dsp = phB.enter_context(tc.tile_pool(name="dsp", bufs=12))
            recp = phB.enter_context(tc.tile_pool(name="recp", bufs=2))
            sps = phB.enter_context(tc.tile_pool(name="sps", bufs=2, space="PSUM"))

            edge2 = edge[:].rearrange("p (u v) -> p u v", u=2)
            EW = IT - 64  # stride between the two edge blocks of a pair

            def attention(hq, fillers=()):
                fillers = list(fillers)

                def pop_filler():
                    if fillers:
                        fillers.pop(0)()

                kvh = hq // G
                qt = qta_box[0][:, hq * S_LOC : (hq + 1) * S_LOC]
                for t in range(NT_I):
                    qsl = qt[:, t * IT : (t + 1) * IT]
                    yps = pB.tile([P, IT], dt.float32, name="yps", tag="yps", bufs=2)
                    dss = []
                    state = {"y_first": True, "pend": [], "defers": 0}

                    def consume_pair(info, last):
                        # y-MMs + denominator pair-sum, emitted two pairs
                        # late so their semaphores clear before the PE
                        # reaches them
                        pr, j_hi, j_lo, ohi, olo = info
                        ds = dsp.tile([P, IT], dt.bfloat16, name="ds")
                        if ohi:  # band pair: ragged union [olo, IT)
                            nc.vector.tensor_copy(
                                ds[:, olo:ohi], pr[:, IT + olo : IT + ohi]
                            )
                            nc.vector.tensor_add(
                                ds[:, ohi:IT], pr[:, ohi:IT], pr[:, IT + ohi : 2 * IT]
                            )
                        else:
                            nc.vector.tensor_add(
                                ds[:, 0:IT], pr[:, 0:IT], pr[:, IT : 2 * IT]
                            )
                        dss.append((ds, olo))
                        nc.tensor.matmul(
                            yps[:, ohi:IT],
                            vv[:, j_hi * KVC + kvh * HD : j_hi * KVC + (kvh + 1) * HD],
                            pr[:, ohi:IT],
                            start=state["y_first"],
                            stop=False,
                        )
                        state["y_first"] = False
                        nc.tensor.matmul(
                            yps[:, olo:IT],
                            vv[:, j_lo * KVC + kvh * HD : j_lo * KVC + (kvh + 1) * HD],
                            pr[:, IT + olo : 2 * IT],
                            start=False,
                            stop=last,
                        )

                    def defer(info):
                        state["pend"].append(info)
                        if len(state["pend"]) > 4:
                            consume_pair(state["pend"].pop(0), last=False)
                        state["defers"] += 1
                        if state["defers"] == FLUSH_AT:
                            # previous (head,t)'s finalize, a couple pairs
                            # into this tile's stream
                            flush_pending()
                        pop_filler()

                    # diagonal band tiles: ragged pairs
                    for bp in range(JB // 2):
                        jlo, jhi = 2 * bp, 2 * bp + 1
                        j_lo, j_hi = t * JB + jlo, t * JB + jhi
                        olo, ohi = 64 * jlo, 64 * jhi
                        sp = sps.tile([P, 2 * IT], dt.float32, name="sps")
                        nc.tensor.matmul(
                            sp[:, ohi:IT],
                            kT[:, kvh * S + j_hi * P : kvh * S + (j_hi + 1) * P],
                            qt[:, t * IT + ohi : (t + 1) * IT],
                            start=True,
                            stop=True,
                        )
                        nc.tensor.matmul(
                            sp[:, IT + olo : 2 * IT],
                            kT[:, kvh * S + j_lo * P : kvh * S + (j_lo + 1) * P],
                            qt[:, t * IT + olo : (t + 1) * IT],
                            start=True,
                            stop=True,
                        )
                        pr = prp.tile([P, 2 * IT], dt.bfloat16, name="pr")
                        # one strided exp covering [olo,IT) and [IT+olo,2IT):
                        # equal-width blocks IT apart; the 64 cols [olo,ohi)
                        # of member0 are unwritten junk that nothing reads
                        nc.scalar.activation(
                            pr[:, 0 : 2 * IT].rearrange(
                                "p (u v) -> p u v", u=2
                            )[:, :, olo:IT],
                            sp[:, 0 : 2 * IT].rearrange(
                                "p (u v) -> p u v", u=2
                            )[:, :, olo:IT],
                            AF.Exp, scale=SCALE,
                        )
                        # both 64-wide edge blocks ([ohi,+64) and [IT+olo,+64),
                        # stride EW apart) in one strided op; X shifts the
                        # window so the 2*EW base slice stays inside the tile
                        X = max(0, ohi - (2 * IT - 2 * EW))
                        eap = pr[:, ohi - X : ohi - X + 2 * EW].rearrange(
                            "p (u v) -> p u v", u=2
                        )[:, :, X : X + 64]
                        nc.gpsimd.tensor_mul(eap, eap, edge2)
                        defer((pr, j_hi, j_lo, ohi, olo))
                    # full past tiles, in pairs (no edge dependency:
                    # they drain the tile cleanly while band edges settle)
                    for jp in range(t * JB // 2):
                        j0 = 2 * jp
                        sp = sps.tile([P, 2 * IT], dt.float32, name="sps")
                        for u in (0, 1):
                            nc.tensor.matmul(
                                sp[:, u * IT : (u + 1) * IT],
                                kT[:, kvh * S + (j0 + u) * P : kvh * S + (j0 + u + 1) * P],
                                qsl,
                                start=True,
                                stop=True,
                            )
                        pr = prp.tile([P, 2 * IT], dt.bfloat16, name="pr")
                        nc.scalar.activation(
                            pr[:, 0 : 2 * IT], sp[:], AF.Exp, scale=SCALE
                        )
                        defer((pr, j0, j0 + 1, 0, 0))

                    while fillers and t == NT_I - 1:
                        fillers.pop(0)()
                    while state["pend"]:
                        consume_pair(
                            state["pend"].pop(0), last=not state["pend"]
                        )

                    def fin(hq=hq, t=t, yps=yps, dss=tuple(dss)):
                        dps = pB.tile([P, IT], dt.float32, name="dps",
                                      tag="dps", bufs=1)
                        for i, (ds, lo) in enumerate(dss):
                            nc.tensor.matmul(
                                dps[:, lo:IT], onesm[:], ds[:, lo:IT],
                                start=(i == 0), stop=(i == len(dss) - 1),
                            )
                        rec = recp.tile([P, IT], dt.float32, name="rec")
                        nc.vector.reciprocal_approx_fast(rec[:], dps[:])
                        nc.vector.tensor_mul(
                            yT[:, hq * S_LOC + t * IT : hq * S_LOC + (t + 1) * IT],
                            yps[:],
                            rec[:],
                        )

                    pending[0] = fin

            # B-side q projection of heads NH_A..15 as 4-matmul quarter
            # groups, interleaved one per pair inside the attention stream:
            # the PE does filler work exactly where it would otherwise
            # wait for exp to free a score-psum buffer.
            def q_quarters(hq, t, u):
                if t == 0 and u == 0:
                    wqt = wqr.tile([P, HT * HD], dt.bfloat16, name="wqt")
                    nc.sync.dma_start(
                        wqt[:].rearrange("p (t m) -> p t m", t=HT),
                        wq_d.ap()[hq],
                    )
                    stg = rpq.tile([P, S_LOC], dt.bfloat16, name="stgq",
                                   tag="stgq", bufs=2)
                    qstate[hq] = [wqt, stg, None]

                def quarter(h0, hq=hq, t=t, u=u):
                    st = qstate[hq]
                    if u == 0 and h0 == 0:
                        st[2] = pB.tile([P, IT], dt.float32, name="qpsB",
                                        tag="qpsB", bufs=1)
                    wqt, stg, ps = st
                    for h in range(u * 8 + h0, u * 8 + h0 + 4):
                        nc.tensor.matmul(
                            ps[:],
                            wqt[:, h * HD : (h + 1) * HD],
                            xq[:, h * S_LOC + t * IT : h * S_LOC + (t + 1) * IT],
                            start=(h == 0),
                            stop=(h == HT - 1),
                        )
                    if u == 1 and h0 == 4:
                        nc.vector.tensor_copy(
                            stg[:, t * IT : (t + 1) * IT], ps[:]
                        )
                        if t == 1:
                            rope_finish(hq)

                return [lambda q=quarter: q(0), lambda q=quarter: q(4)]

            qb = [
                (NH_A + i // 4, (i // 2) % 2, i % 2)
                for i in range((NH - NH_A) * 4)
            ]
            # two half-groups (4 quarters) per slot until exhausted; done
            # by slot 11 so the w_proj prefetch gets the freed xq space
            nxt = 0
            for s in range(NH):
                fillers = []
                for _ in range(2):
                    if nxt < len(qb):
                        fillers += q_quarters(*qb[nxt])
                        nxt += 1
                attention(s, fillers)
                if nxt == len(qb) and not wpa_box:
                    # xq / wq streams are done: release their SBUF and
                    # start the w_proj prefetch in the freed region
                    s_q.close()
                    wpap = body.enter_context(
                        tc.tile_pool(name="wpap", bufs=1, side="right")
                    )
                    wpa_box.append(
                        wpap.tile([P, WPA * H], dt.bfloat16, name="wpa")
                    )
                    for i in range(WPA):
                        nc.sync.dma_start(
                            wpa_box[0][:, i * H : (i + 1) * H], wp_d.ap()[i]
                        )
            flush_pending()
            wpa = wpa_box[0]
        s_ab.close()  # free kT / vv / qta before the projection phase

        # ---------------- phase C: out projection + rmsnorm -------------
        with ExitStack() as phC:
            wpbp = phC.enter_context(tc.tile_pool(name="wpbp", bufs=1))
            wpc = wpbp.tile([P, (HT - WPA) * H], dt.bfloat16)
            for i in range(HT - WPA):
                nc.sync.dma_start(
                    wpc[:, i * H : (i + 1) * H], wp_d.ap()[WPA + i]
                )
            nwp = phC.enter_context(tc.tile_pool(name="nwp", bufs=1))
            nw = nwp.tile([P, H], dt.float32)
            nc.sync.dma_start(nw[:], nw_d.ap())

            outp = phC.enter_context(tc.tile_pool(name="outp", bufs=3))
            sqp = phC.enter_context(tc.tile_pool(name="sqp", bufs=3))
            smp = phC.enter_context(tc.tile_pool(name="smp", bufs=2))
            po = phC.enter_context(tc.tile_pool(name="po", bufs=8, space="PSUM"))

            mult = mybir.AluOpType.mult

            nslice = S_LOC // P
            for sl in range(nslice):
                # last slice runs o-outer so the norm chain pipelines with
                # the matmuls and the tail after the final matmul is short
                o_outer = sl == nslice - 1
                pso = [po.tile([P, OT], dt.float32, name="pso") for _ in range(NO)]
                ot = outp.tile([P, H], dt.float32, name="ot")
                ssqs = []

                def chunk_post(o):
                    sq = sqp.tile([P, OT], dt.float32, name="sq")
                    sso = smp.tile([P, 1], dt.float32, name="sso", tag="sso", bufs=8)
                    nc.scalar.activation(
                        sq[:], pso[o][:], AF.Square, accum_out=sso[:]
                    )
                    nc.scalar.activation(
                        ot[:, o * OT : (o + 1) * OT], pso[o][:], AF.Copy
                    )
                    ssqs.append(sso)

                lhss = [
                    yT[:, h * S_LOC + sl * P : h * S_LOC + (sl + 1) * P]
                    for h in range(HT)
                ]
                def wslice(h, o):
                    if h < WPA:
                        return wpa[:, h * H + o * OT : h * H + (o + 1) * OT]
                    hh = h - WPA
                    return wpc[:, hh * H + o * OT : hh * H + (o + 1) * OT]

                if o_outer:
                    for o in range(NO):
                        for h in range(HT):
                            nc.tensor.matmul(
                                pso[o][:],
                                lhss[h],
                                wslice(h, o),
                                start=(h == 0),
                                stop=(h == HT - 1),
                            )
                        chunk_post(o)
                else:
                    for h in range(HT):
                        for o in range(NO):
                            nc.tensor.matmul(
                                pso[o][:],
                                lhss[h],
                                wslice(h, o),
                                start=(h == 0),
                                stop=(h == HT - 1),
                            )
                    for o in range(NO):
                        chunk_post(o)
                sa = smp.tile([P, 1], dt.float32, name="sa")
                sb = smp.tile([P, 1], dt.float32, name="sb")
                nc.vector.tensor_add(sa[:], ssqs[0][:], ssqs[1][:])
                nc.vector.tensor_add(sb[:], ssqs[2][:], ssqs[3][:])
                ssq = smp.tile([P, 1], dt.float32, name="ssq")
                nc.vector.tensor_add(ssq[:], sa[:], sb[:])
                rms = smp.tile([P, 1], dt.float32, name="rms")
                nc.scalar.activation(
                    rms[:], ssq[:], AF.Sqrt, bias=epsb[:], scale=1.0 / H
                )
                rr = smp.tile([P, 1], dt.float32, name="rr")
                nc.vector.reciprocal(rr[:], rms[:])
                for half in range(2):
                    for o in (2 * half, 2 * half + 1):
                        nc.vector.scalar_tensor_tensor(
                            ot[:, o * OT : (o + 1) * OT],
                            ot[:, o * OT : (o + 1) * OT],
                            rr[:],
                            nw[:, o * OT : (o + 1) * OT],
                            mult,
                            mult,
                        )
                    nc.sync.dma_start(
                        out_d.ap()[sl * P : (sl + 1) * P, half * H // 2 : (half + 1) * H // 2],
                        ot[:, half * H // 2 : (half + 1) * H // 2],
                    )

    nc.compile()
    return nc


# ---------------------------------------------------------------- host side
def _host_shared(w_attn, w_proj, norm_w):
    """Core-independent packed tensors."""
    f32 = np.float32

    def perm_halves(w):  # [H, n, HD] even/odd pairs -> halves
        return np.concatenate([w[..., 0::2], w[..., 1::2]], axis=-1)

    wq = perm_halves(w_attn[:, :H].reshape(H, NH, HD))
    wq = np.ascontiguousarray(
        wq.reshape(HT, P, NH, HD).transpose(2, 1, 0, 3)
    ).astype(BF16)
    wk = perm_halves(w_attn[:, H : H + KVC].reshape(H, NKV, HD))
    wk = np.ascontiguousarray(
        wk.reshape(HT, P, NKV, HD).transpose(2, 1, 0, 3)
    ).astype(BF16)
    wv = np.ascontiguousarray(
        w_attn[:, H + KVC :].reshape(HT, P, KVC)
    ).astype(BF16)
    wp = np.ascontiguousarray(w_proj.reshape(HT, P, H)).astype(BF16)

    p, f = np.meshgrid(np.arange(P), np.arange(64), indexing="ij")
    # parity 0: query col f = global row 2f vs key p (natural order)
    edge0 = (2 * f >= p).astype(BF16)
    # parity 1: query 2f+1 vs key p
    edge1 = (2 * f + 1 >= p).astype(BF16)
    # duplicated side by side: one strided op masks both blocks of a pair
    edge0 = np.ascontiguousarray(np.concatenate([edge0, edge0], axis=1))
    edge1 = np.ascontiguousarray(np.concatenate([edge1, edge1], axis=1))

    nw = np.ascontiguousarray(
        np.broadcast_to(norm_w.astype(f32), (P, H))
    )
    return wq, wk, wv, wp, (edge0, edge1), nw


def _cos_sin(pos):
    f32 = np.float32
    inv = 1.0 / (
        10000.0 ** (np.arange(0, HD, 2, dtype=f32) / f32(HD))
    )
    ang = inv[:, None].astype(f32) * pos[None, :].astype(f32)  # [HH, N]
    c, s = np.cos(ang).astype(BF16), np.sin(ang).astype(BF16)
    # duplicated across both partition halves (walrus wants equal base
    # partitions for SBUF tensor-tensor inputs)
    return (
        np.ascontiguousarray(np.concatenate([c, c], axis=0)),
        np.ascontiguousarray(np.concatenate([s, s], axis=0)),
    )


def make_in_maps(x, w_attn, w_proj, norm_w):
    x = np.asarray(x, dtype=np.float32)
    w_attn = np.asarray(w_attn, dtype=np.float32)
    w_proj = np.asarray(w_proj, dtype=np.float32)
    norm_w = np.asarray(norm_w, dtype=np.float32)

    wq, wk, wv, wp, (edge0, edge1), nw = _host_shared(w_attn, w_proj, norm_w)

    kc, ks = _cos_sin(np.arange(S, dtype=np.float32))
    qc0, qs0 = _cos_sin(2.0 * np.arange(S_LOC, dtype=np.float32))
    qc1, qs1 = _cos_sin(2.0 * np.arange(S_LOC, dtype=np.float32) + 1.0)

    in_maps = []
    for c in range(N_CORES):
        b, par = c // 2, c % 2
        xt = x[b].T.astype(BF16)
        # parity-packed contiguous copy for the q projection
        xq = np.ascontiguousarray(
            xt[:, par::2].reshape(HT, P, S_LOC)
        )
        xt = np.ascontiguousarray(xt.reshape(HT, P, S))
        if DEDUP:
            wk_c = np.ascontiguousarray(wk[2 * par : 2 * par + 2])
            wv_c = np.ascontiguousarray(
                wv[:, :, par * KVCL : (par + 1) * KVCL].transpose(1, 0, 2)
            )
        else:
            wk_c = wk
            wv_c = np.ascontiguousarray(wv.transpose(1, 0, 2))
        in_maps.append(
            {
                "xt": xt,
                "xq": xq,
                "wq": wq,
                "wk": wk_c,
                "wv": wv_c,
                "wp": wp,
                "qcos": qc1 if par else qc0,
                "qsin": qs1 if par else qs0,
                "kcos": kc,
                "ksin": ks,
                "edge": edge1 if par else edge0,
                "nw": nw,
            }
        )
    return in_maps


def assemble_out(results):
    out = np.empty((B, S, H), dtype=np.float32)
    for c in range(N_CORES):
        b, par = c // 2, c % 2
        out[b, par::2, :] = results[c]["out"]
    return out


def kernel(x, w_attn, w_proj, norm_w):
    from concourse import bass_utils

    if "nc" not in _CACHE:
        _CACHE["nc"] = _build_nc()
    nc = _CACHE["nc"]

    in_maps = make_in_maps(x, w_attn, w_proj, norm_w)
    res = bass_utils.run_bass_kernel_spmd(
        nc, in_maps, core_ids=list(range(N_CORES))
    )
    return assemble_out(res.results)
